# revision 14
# baseline (speedup 1.0000x reference)
"""3-layer GCN on 8 Trainium2 NeuronCores (Bass/Tile).

Sharding: nodes are partitioned contiguously across the 8 cores (graph
parallel). Messages (edges + self loops) are bucketed by destination core,
gathered from a replicated node-feature table in HBM via dma_gather,
segment-summed on the PE via one-hot selection matmuls, and the per-layer
dense transforms produce the next layer's table shard which is AllGathered.

Math (b = bias, dinv = D^-1/2 incl. self loops, A = adjacency + self loops):
  layer:  y = dinv * (A @ (dinv * (in @ W))) + b
  layer1 is reordered as y1 = (dinv * (A @ (dinv * x))) @ W1 + b1 so the
  first gather table (dinv*x) is pure input layout prep.
"""

import os
import numpy as np
import ml_dtypes

import concourse.bacc as bacc
import concourse.mybir as mybir
import concourse.tile as tile
from concourse.bass_utils import run_bass_kernel_spmd

BF16 = mybir.dt.bfloat16
F32 = mybir.dt.float32
I16 = mybir.dt.int16

N_NODES = 50000
NCORES = 8
F_IN = 128
F_HID = 128
F_OUT = 64  # padded to 128 on device

WIN = 64          # dests per selection window
WPV = 8           # windows per wave (psum tile = 512 dests)
HPW = 6           # waves per half-pass (psum pool bufs)
MAXCH = 64        # max chunks (of 128 msgs) per dma_gather call
NQUEUES = 1       # SWDGE queues used for gathers

LAST_RESULT = {}  # test harness introspection (exec_time_ns etc.)


def _ceil(a, b):
    return (a + b - 1) // b


def _layout(edge_index, n_nodes, ncores):
    """Host-side structural preprocessing. Returns layout dict."""
    npc = n_nodes // ncores
    half = n_nodes // 2
    assert n_nodes % ncores == 0 and half <= 32768
    nw = _ceil(npc, WIN)              # windows per core
    nv = _ceil(nw, WPV)               # waves per core
    nt = _ceil(npc, 128)              # dense node tiles per core

    ei = np.asarray(edge_index)
    loops = np.arange(n_nodes, dtype=np.int64)
    row = np.concatenate([ei[0], loops])
    col = np.concatenate([ei[1], loops])
    deg = np.bincount(col, minlength=n_nodes).astype(np.float32)
    dinv = np.where(deg > 0, 1.0 / np.sqrt(deg), 0.0).astype(np.float32)

    owner = (col // npc).astype(np.int64)
    dl = (col - owner * npc).astype(np.int64)
    win = dl // WIN
    hlf = (row >= half).astype(np.int64)
    cloc = (dl - win * WIN).astype(np.float32)
    src = np.where(hlf == 0, row, row - half).astype(np.int16)

    key = (owner * nw + win) * 2 + hlf
    counts = np.bincount(key, minlength=ncores * nw * 2).reshape(ncores, nw, 2)
    ch = _ceil(counts.max(axis=0), 128)          # [nw, 2] chunks per segment
    empty = ch.sum(axis=1) == 0
    ch[empty, 0] = 1

    # wave / half-pass structure
    waves = [range(v * WPV, min((v + 1) * WPV, nw)) for v in range(nv)]
    hps = [range(p * HPW, min((p + 1) * HPW, nv)) for p in range(_ceil(nv, HPW))]

    # stream order of segments, chunk bases
    seg_order = []
    for hp in hps:
        for h in (0, 1):
            for v in hp:
                for w in waves[v]:
                    seg_order.append((w, h))
    chunk_base = {}
    totc = 0
    for w, h in seg_order:
        chunk_base[(w, h)] = totc
        totc += ch[w, h]
    tots = totc * 128

    # per-chunk window + start/stop flags
    chunk_win = np.empty(totc, dtype=np.int64)
    for w, h in seg_order:
        b = chunk_base[(w, h)]
        chunk_win[b:b + ch[w, h]] = w
    first = {}
    last = {}
    for c in range(totc):
        w = int(chunk_win[c])
        if w not in first:
            first[w] = c
        last[w] = c
    chunk_start = np.zeros(totc, dtype=bool)
    chunk_stop = np.zeros(totc, dtype=bool)
    for w in first:
        chunk_start[first[w]] = True
        chunk_stop[last[w]] = True

    # gather calls: split each (hp, half) chunk range into pieces <= MAXCH
    calls = []
    q = 0
    for hp in hps:
        for h in (0, 1):
            ws = [w for v in hp for w in waves[v]]
            c0 = chunk_base[(ws[0], h)]
            c1 = chunk_base[(ws[-1], h)] + ch[ws[-1], h]
            c = c0
            while c < c1:
                n = min(MAXCH, c1 - c)
                calls.append(dict(c0=c, n=n, half=h, queue=q % NQUEUES))
                q += 1
                c += n

    # per-core slot data
    src_slots = np.zeros((ncores, tots), dtype=np.int16)
    col_slots = np.full((ncores, tots), -1.0, dtype=np.float32)
    seg_slot_base = np.zeros((nw, 2), dtype=np.int64)
    for w, h in seg_order:
        seg_slot_base[w, h] = chunk_base[(w, h)] * 128
    for c in range(ncores):
        m = owner == c
        k = (win[m] * 2 + hlf[m]).astype(np.int64)
        order = np.argsort(k, kind="stable")
        ks = k[order]
        seg_counts = np.bincount(ks, minlength=nw * 2)
        starts = np.concatenate([[0], np.cumsum(seg_counts)[:-1]])
        within = np.arange(ks.size) - starts[ks]
        base = seg_slot_base.reshape(-1)[ks]
        slot = base + within
        src_slots[c, slot] = src[m][order]
        col_slots[c, slot] = cloc[m][order]

    # packed per-core gather indices (concatenated per call) and col-locals
    gidx = np.zeros((ncores, 128, tots // 16), dtype=np.int16)
    for c in range(ncores):
        off = 0
        for cl in calls:
            s0, n = cl["c0"] * 128, cl["n"] * 128
            blk = src_slots[c, s0:s0 + n].reshape(-1, 16).T  # [16, n/16]
            gidx[c, :, off:off + n // 16] = np.tile(blk, (8, 1))
            off += n // 16
        cl_off = 0
        for cl in calls:
            cl["idx_off"] = cl_off
            cl_off += cl["n"] * 8  # columns of int16 per call
    colloc = col_slots.reshape(ncores, totc, 128).transpose(0, 2, 1).copy()

    return dict(
        npc=npc, half=half, nw=nw, nv=nv, nt=nt, totc=totc, tots=tots,
        dinv=dinv, ch=ch, waves=waves, hps=hps, chunk_base=chunk_base,
        chunk_win=chunk_win, chunk_start=chunk_start, chunk_stop=chunk_stop,
        calls=calls, gidx=gidx, colloc=colloc, seg_order=seg_order,
        n_nodes=n_nodes, ncores=ncores,
    )


def _emit_layer(nc, lay, L, tabs, params, pools, meta):
    """Emit one GCN layer (aggregation + epilogues + dense + collective)."""
    npc, nw, nv = meta["npc"], meta["nw"], meta["nv"]
    waves, hps = meta["waves"], meta["hps"]
    ch, chunk_base = meta["ch"], meta["chunk_base"]
    chunk_win, chunk_start, chunk_stop = (
        meta["chunk_win"], meta["chunk_start"], meta["chunk_stop"])
    calls, half_rows = meta["calls"], meta["half"]

    (msgp, selp, aggp, dnsp, zp, stgp) = (
        pools["msg"], pools["sel"], pools["agg"], pools["dns"], pools["z"],
        pools["stg"])
    gidx_t, colloc_t, dinvrep_t, dinvnode_t, iota_t = (
        params["gidx"], params["colloc"], params["dinvrep"],
        params["dinvnode"], params["iota"])
    w_t = [None, params["w1"], params["w2"], params["w3"]]
    b_t = [None, params["b1"], params["b2"], params["b3"]]

    table = tabs["t%d" % L]
    ncols = [0, F_HID, F_HID, 128][L]
    sub = int(os.environ.get("GCN_SUB", "6"))

    # --- gather calls (all emitted up-front; Tile schedules by deps) ---
    msg_tiles = {}
    for cl in calls:
        mt = msgp.tile([128, cl["n"], 128], BF16, tag="msg", name="msg")
        nidx = cl["n"] * 128
        src_ap = table[cl["half"] * half_rows:(cl["half"] + 1) * half_rows, :]
        idx_ap = gidx_t[:, cl["idx_off"]:cl["idx_off"] + cl["n"] * 8]
        nc.gpsimd.dma_gather(mt[:], src_ap, idx_ap, nidx, nidx, 128,
                             queue_num=cl["queue"], single_packet=False)
        for i in range(cl["n"]):
            msg_tiles[cl["c0"] + i] = (mt, i)
    if sub <= 1:
        return

    # --- selection tiles + chunk matmuls, wave epilogues, dense ---
    wave_psum = {}
    wave_left = {}
    for v in range(nv):
        wave_left[v] = int(sum(ch[w, :].sum() for w in waves[v]))
    sel_tiles = {}

    def wave_cols(v):
        lo = waves[v][0] * WIN
        hi = min((waves[v][-1] + 1) * WIN, nw * WIN)
        return lo, hi - lo

    dense_done = [0]

    def emit_dense_upto(cols_done, rsrc):
        """dense tiles fully covered by r columns so far (L in (1,2))."""
        w_next = w_t[L + 1]
        shard = tabs["s%d" % (L + 1)]
        while (dense_done[0] + 1) * 128 <= cols_done and dense_done[0] < meta["nt"]:
            k = dense_done[0]
            dps = dnsp.tile([128, 512], F32, tag="dns", space="PSUM", name="dns")
            nc.tensor.matmul(dps[:, :128], rsrc[:, k * 128:(k + 1) * 128],
                             w_next[:], start=True, stop=True)
            stg = stgp.tile([128, 128], BF16, tag="stg", name="stg")
            nc.scalar.activation(stg[:], dps[:, :128],
                                 mybir.ActivationFunctionType.Identity,
                                 scale=dinvnode_t[:, k:k + 1])
            rows = min(128, npc - k * 128)
            nc.sync.dma_start(shard[k * 128:k * 128 + rows, :], stg[:rows, :])
            dense_done[0] += 1

    zs = zp.tile([128, nw * WIN], BF16, tag="zs", name="zs")
    if L < 3:
        r = zp.tile([128, nw * WIN], BF16, tag="r", name="r")
    else:
        y3 = params["y3_tile"]

    for c in range(meta["totc"]):
        b8 = c // 8
        if b8 not in sel_tiles:
            bsz = min(8, meta["totc"] - b8 * 8)
            st = selp.tile([128, 8, WIN], BF16, tag="sel", name="sel")
            nc.vector.tensor_tensor(
                out=st[:, :bsz, :],
                in0=colloc_t[:, b8 * 8:b8 * 8 + bsz].to_broadcast(
                    [128, bsz, WIN]),
                in1=iota_t[:, :bsz, :],
                op=mybir.AluOpType.is_equal)
            sel_tiles[b8] = st
        if sub <= 2:
            continue
        w = int(chunk_win[c])
        v = w // WPV
        if v not in wave_psum:
            wave_psum[v] = aggp.tile([128, 512], F32, tag="agg", space="PSUM", name="aggps")
            _, zn = wave_cols(v)
            nc.tensor.matmul(wave_psum[v][:, :zn], params["zrow"][:1, :128],
                             params["zrow"][:1, :zn], start=True, stop=False,
                             skip_group_check=True)
        mt, mi = msg_tiles[c]
        wl = w - waves[v][0]
        nc.tensor.matmul(
            wave_psum[v][:, wl * WIN:(wl + 1) * WIN],
            mt[:, mi, :ncols] if ncols < 128 else mt[:, mi, :],
            sel_tiles[b8][:, c - b8 * 8, :],
            start=False, stop=False, skip_group_check=True)

        wave_left[v] -= 1
        if wave_left[v] == 0 and sub >= 4:
            lo, n = wave_cols(v)
            ps = wave_psum.pop(v)
            if L < 3:
                # z_scaled = psum * dinv_rep  -> bf16
                nc.vector.tensor_tensor(
                    out=zs[:, lo:lo + n], in0=ps[:, :n],
                    in1=dinvrep_t[:, lo:lo + n], op=mybir.AluOpType.mult)
                if L == 1:
                    # y1 = zs @ W1 (feature-major): lhsT=W1, rhs=zs
                    dps = dnsp.tile([128, 512], F32, tag="dns", space="PSUM", name="dns")
                    nc.tensor.matmul(dps[:, :n], w_t[1][:], zs[:, lo:lo + n],
                                     start=True, stop=True)
                    nc.scalar.activation(r[:, lo:lo + n], dps[:, :n],
                                         mybir.ActivationFunctionType.Relu,
                                         bias=b_t[1][:])
                else:
                    nc.scalar.activation(r[:, lo:lo + n], zs[:, lo:lo + n],
                                         mybir.ActivationFunctionType.Relu,
                                         bias=b_t[2][:])
                if sub >= 5:
                    emit_dense_upto(lo + n, r)
            else:
                tmp = stgp.tile([64, 512], F32, tag="tmp3", name="tmp3")
                nc.vector.tensor_tensor(
                    out=tmp[:, :n], in0=ps[:64, :n],
                    in1=dinvrep_t[:64, lo:lo + n], op=mybir.AluOpType.mult)
                nc.scalar.activation(y3[:, lo:lo + n], tmp[:, :n],
                                     mybir.ActivationFunctionType.Identity,
                                     bias=b_t[3][:64, :])

    dump = os.environ.get("GCN_DUMP", "")
    if dump and int(os.environ.get("GCN_DUMPL", "1")) == L:
        t = {"zs": zs, "r": (r if L < 3 else zs)}[dump]
        nc.gpsimd.dma_start(params["dbg_p"][:], t[:])
        params["dbg_written"] = True
    if L < 3 and sub >= 6:
        shard = tabs["s%d" % (L + 1)]
        full = tabs["t%d" % (L + 1)]
        nc.gpsimd.collective_compute(
            "AllGather", mybir.AluOpType.bypass,
            replica_groups=[list(range(meta["ncores"]))],
            ins=[shard[:]], outs=[full[:]])


def _build_program(meta):
    npc, n_nodes = meta["npc"], meta["n_nodes"]
    nc = bacc.Bacc("TRN2", num_swdge_queues=NQUEUES)

    p = {}
    p["table1_p"] = nc.declare_dram_parameter("table1", [n_nodes, 128], BF16,
                                              isOutput=False)
    gidx_p = nc.declare_dram_parameter(
        "gidx", [128, meta["tots"] // 16], I16, isOutput=False)
    colloc_p = nc.declare_dram_parameter(
        "colloc", [128, meta["totc"]], F32, isOutput=False)
    dinvrep_p = nc.declare_dram_parameter(
        "dinvrep", [128, meta["nw"] * WIN], F32, isOutput=False)
    dinvnode_p = nc.declare_dram_parameter(
        "dinvnode", [128, meta["nt"]], F32, isOutput=False)
    iota_p = nc.declare_dram_parameter("iota", [128, 8, WIN], F32,
                                       isOutput=False)
    zrow_p = nc.declare_dram_parameter("zrow", [1, 512], BF16, isOutput=False)
    w_p = {k: nc.declare_dram_parameter(k, [128, 128], BF16, isOutput=False)
           for k in ("w1", "w2", "w3")}
    b_p = {k: nc.declare_dram_parameter(k, [128, 1], F32, isOutput=False)
           for k in ("b1", "b2", "b3")}
    out_p = nc.declare_dram_parameter("y3t", [F_OUT, npc], F32, isOutput=True)
    dbg_p = nc.declare_dram_parameter("dbg", [128, meta["nw"] * WIN], F32,
                                      isOutput=True)

    tabs = {"t1": p["table1_p"]}
    for L in (2, 3):
        tabs["s%d" % L] = nc.dram_tensor("s%d" % L, [npc, 128], BF16)
        tabs["t%d" % L] = nc.dram_tensor("t%d" % L, [n_nodes, 128], BF16,
                                         addr_space="Shared")

    with tile.TileContext(nc) as tc:
        with (
            tc.tile_pool(name="const", bufs=1) as constp,
            tc.tile_pool(name="msg", bufs=int(os.environ.get("GCN_MSGBUFS", "3"))) as msgp,
            tc.tile_pool(name="sel", bufs=6) as selp,
            tc.tile_pool(name="agg", bufs=HPW, space="PSUM") as aggp,
            tc.tile_pool(name="dns", bufs=2, space="PSUM") as dnsp,
            tc.tile_pool(name="z", bufs=1) as zp,
            tc.tile_pool(name="stg", bufs=3) as stgp,
        ):
            params = {}
            for nm, src, shp, dt in (
                ("gidx", gidx_p, [128, meta["tots"] // 16], I16),
                ("colloc", colloc_p, [128, meta["totc"]], F32),
                ("dinvrep", dinvrep_p, [128, meta["nw"] * WIN], F32),
                ("dinvnode", dinvnode_p, [128, meta["nt"]], F32),
                ("iota", iota_p, [128, 8, WIN], F32),
                ("zrow", zrow_p, [1, 512], BF16),
                ("w1", w_p["w1"], [128, 128], BF16),
                ("w2", w_p["w2"], [128, 128], BF16),
                ("w3", w_p["w3"], [128, 128], BF16),
                ("b1", b_p["b1"], [128, 1], F32),
                ("b2", b_p["b2"], [128, 1], F32),
                ("b3", b_p["b3"], [128, 1], F32),
            ):
                t = constp.tile(shp, dt, tag=nm, name=nm)
                nc.sync.dma_start(t[:], src[:])
                params[nm] = t
            params["y3_tile"] = constp.tile([F_OUT, meta["nw"] * WIN], F32,
                                            tag="y3", name="y3")
            params["dbg_p"] = dbg_p

            pools = dict(msg=msgp, sel=selp, agg=aggp, dns=dnsp, z=zp,
                         stg=stgp)
            stage = int(os.environ.get("GCN_STAGE", "3"))

            for L in (1, 2, 3)[:stage]:
                _emit_layer(nc, None, L, tabs, params, pools, meta)

            if stage == 3:
                nc.sync.dma_start(out_p[:], params["y3_tile"][:, :npc])
            else:
                nc.sync.dma_start(out_p[:], params["dinvrep"][:F_OUT, :npc])
            if not params.get("dbg_written"):
                nc.sync.dma_start(dbg_p[:], params["dinvrep"][:])

    nc.compile()
    return nc


def _prep_inputs(x, W1, b1, W2, b2, W3, b3, meta):
    dinv = meta["dinv"]
    n_nodes, ncores, npc = meta["n_nodes"], meta["ncores"], meta["npc"]
    table1 = (dinv[:, None] * np.asarray(x, np.float32)).astype(
        ml_dtypes.bfloat16)
    w3p = np.zeros((128, 128), np.float32)
    w3p[:, :F_OUT] = np.asarray(W3, np.float32)
    b3p = np.zeros((128,), np.float32)
    b3p[:F_OUT] = np.asarray(b3, np.float32)
    iota = np.tile(np.arange(WIN, dtype=np.float32), (128, 8, 1))

    in_maps = []
    for c in range(ncores):
        dl = dinv[c * npc:(c + 1) * npc]
        drep = np.zeros((128, meta["nw"] * WIN), np.float32)
        drep[:, :npc] = dl[None, :]
        dnode = np.zeros((128, meta["nt"]), np.float32)
        dpad = np.zeros(meta["nt"] * 128, np.float32)
        dpad[:npc] = dl
        dnode[:, :] = dpad.reshape(meta["nt"], 128).T
        in_maps.append({
            "table1": table1,
            "gidx": meta["gidx"][c],
            "colloc": meta["colloc"][c],
            "dinvrep": drep,
            "dinvnode": dnode,
            "iota": iota,
            "zrow": np.zeros((1, 512), ml_dtypes.bfloat16),
            "w1": np.asarray(W1, np.float32).astype(ml_dtypes.bfloat16),
            "w2": np.asarray(W2, np.float32).astype(ml_dtypes.bfloat16),
            "w3": w3p.astype(ml_dtypes.bfloat16),
            "b1": np.asarray(b1, np.float32).reshape(128, 1),
            "b2": np.asarray(b2, np.float32).reshape(128, 1),
            "b3": b3p.reshape(128, 1),
        })
    return in_maps


def run_gcn(x, edge_index, W1, b1, W2, b2, W3, b3,
            n_nodes=N_NODES, ncores=NCORES, trace=False):
    meta = _layout(edge_index, n_nodes, ncores)
    nc = _build_program(meta)
    in_maps = _prep_inputs(x, W1, b1, W2, b2, W3, b3, meta)
    res = run_bass_kernel_spmd(nc, in_maps, list(range(ncores)), trace=trace)
    LAST_RESULT["exec_time_ns"] = getattr(res, "exec_time_ns", None)
    LAST_RESULT["res"] = res
    npc = meta["npc"]
    shards = [np.asarray(res.results[c]["y3t"]) for c in range(ncores)]
    y = np.concatenate(shards, axis=1).T.astype(np.float32)
    return y


def kernel(x, edge_index, W1, b1, W2, b2, W3, b3):
    return run_gcn(x, edge_index, W1, b1, W2, b2, W3, b3,
                   trace=bool(os.environ.get("GCN_TRACE")))
